# revision 25
# baseline (speedup 1.0000x reference)
"""Multi-head attention (bs=2, heads=8, ch=64, len=4096) on 8 Trainium2 cores.

Sharding: bs*heads = 16 head-problems, 2 per core (head/data parallel,
no cross-core communication).

Per-core algorithm (per head, seq len L=4096, ch=64):
  - S^T tiles: st[s,t] = sum_c K[c,s] Q[c,t] on PE in fp32r (no input
    casts; exact), heads packed in array row-halves via tile_position.
  - softmax exp: split across TWO engines to break the ScalarE-only
    roofline (33.5M exps/core = 218us at 1 elem/lane/cycle):
      * ~73% of windows: ScalarE ACT exp (exact, fp32->fp16)
      * ~27% of windows: DVE+Pool 2-phase Schraudolph bit-trick:
          i1 = rint(st*A + B) int16      (DVE tensor_scalar, PSUM read)
          i2 = i1 + K_SHIFT              (DVE tensor_scalar, int16)
          ew = bc16(i1) + bc16(i2)       (Pool tensor_tensor add, fp16)
        With K_SHIFT=-504, phase-2 lands half an octave off with weight
        2^-0.5, flattening the piecewise-linear 2^u error sawtooth:
        rel err std 0.54% (vs 1.77% single-phase), mean calibrated ~0
        via C_CAL so ACT-exact and DVE-approx windows agree in scale.
  - PV with folded denominator: lhsT = [V^T ; ones] fp16, accumulated
    over 32 s-tiles into PSUM [65, 512]; row 64 = softmax denominator.
  - normalize: broadcast denominator FIRST (Pool partition_broadcast),
    then wide [64,512] DVE reciprocal + Pool multiply (a [1,512]
    single-partition reciprocal costs 3.3us on DVE; wide costs 0.7us).

Engine budget per core (steady state): ACT ~185us, DVE ~185us
(exp share + tails), Pool ~160us (combines + tails), PE ~164us@2.4GHz.
"""

import sys

sys.path.insert(0, "/opt/trn_rl_repo")

import numpy as np
from concourse import mybir, tile, bacc
from concourse.bass_utils import run_bass_kernel_spmd
from concourse.masks import make_identity
from concourse.tile_rust import add_dep_helper

dt = mybir.dt

NUM_HEADS = 8
BS = 2
CH = 64
L = 4096
NCORES = 8
HPC = BS * NUM_HEADS // NCORES  # heads per core = 2
NT = 512  # t-chunk (matmul moving dim / PSUM bank)
TCH = L // NT  # 8 t-chunks
NJ = L // 128  # 32 s-tiles
WSZ = 2  # s-tiles per exp window (2 PSUM banks; stp bufs=3 -> 6 banks)
SCALE = float(CH) ** -0.5  # logit scale, folded into ACT / A_TS

# 2-phase Schraudolph constants (calibrated: rel std 0.54%, mean ~0)
LOG2E = 1.4426950408889634
A_TS = SCALE * LOG2E * 1024.0
C_CAL = 853.0
B_TS = 15360.0 - C_CAL
K_SHIFT = -504

# window assignment: DVE path gets F_NUM of every F_DEN windows
F_NUM, F_DEN = 6, 64
PV_LAG = 4  # windows of slack between exp emission and PV consumption

_nc_cache: dict = {}


def _dve_win(g: int) -> bool:
    return (g * F_NUM) % F_DEN < F_NUM


def _build(repeat: int = 1):
    nc = bacc.Bacc("TRN2", target_bir_lowering=False)
    q_d = nc.dram_tensor("q", [HPC * CH, L], dt.float32, kind="ExternalInput")
    k_d = nc.dram_tensor("k", [HPC * CH, L], dt.float32, kind="ExternalInput")
    v_d = nc.dram_tensor("v", [HPC * CH, L], dt.float32, kind="ExternalInput")
    o_d = nc.dram_tensor("o", [HPC * CH, L], dt.float32, kind="ExternalOutput")

    windows = []
    j = 0
    while j < NJ:
        windows.append((j, min(WSZ, NJ - j)))
        j += WSZ

    # Chain every TensorE instruction in emission order (sync=False =
    # order-only, no semaphore): the Tile scheduler otherwise groups
    # same-row-group K=64 matmuls, which serialize (LDWEIGHTS cannot be
    # pulled ahead); the emitted order keeps A/B pairs overlapping.
    prev_pe = [None]

    def chain_pe(bi):
        if prev_pe[0] is not None:
            add_dep_helper(bi.ins, prev_pe[0].ins, sync=False, reason="pe order")
        prev_pe[0] = bi
        return bi

    with tile.TileContext(nc) as tc:
        with (
            tc.tile_pool(name="singles", bufs=1) as singles,
            tc.tile_pool(name="expw", bufs=12) as expp,
            tc.tile_pool(name="i16w", bufs=8) as i16p,
            tc.tile_pool(name="outp", bufs=5) as outp,
            tc.tile_pool(name="tails", bufs=4) as tails,
            tc.tile_pool(name="stp", bufs=3, space="PSUM") as stp,
            tc.tile_pool(name="pvp", bufs=2, space="PSUM") as pvp,
        ):
            ident = singles.tile([128, 128], dt.float32)
            make_identity(nc, ident[:])

            # K and Q packed on the partition axis (head h occupies
            # partitions 64h..64h+63), cast to fp16 for the PE. DMA and
            # cast in column pieces so the first QK matmuls start before
            # the full tensors arrive. All input DMAs move [128, N]
            # tiles ([64, N] transfers get half the SBUF port BW).
            k_r = singles.tile([128, L], dt.float16)
            q_r = singles.tile([128, L], dt.float16)
            qs0 = singles.tile([128, 512], dt.float32)
            ks0 = singles.tile([128, 1024], dt.float32)
            ks1 = singles.tile([128, 1024], dt.float32)
            nc.sync.dma_start(qs0[:], q_d[:, 0:512])
            nc.sync.dma_start(ks0[:], k_d[:, 0:1024])
            nc.sync.dma_start(ks1[:], k_d[:, 1024:2048])
            nc.vector.tensor_copy(q_r[:, 0:512], qs0[:])
            nc.vector.tensor_copy(k_r[:, 0:1024], ks0[:])
            nc.vector.tensor_copy(k_r[:, 1024:2048], ks1[:])

            # V loads on the sync queue ahead of the bulk q/k halves
            # (needed from ~10us for the PE transposes).
            v_f = singles.tile([128, L], dt.float32)
            nc.sync.dma_start(v_f[:, 0:2048], v_d[:, 0:2048])
            nc.sync.dma_start(v_f[:, 2048:L], v_d[:, 2048:L])

            half = L // 2
            raws = {}
            for part in range(2):
                csl = slice(half * part, half * (part + 1))
                eng = nc.sync if part == 0 else nc.gpsimd
                for nm, src in (("q", q_d), ("k", k_d)):
                    if nm == "k" and part == 0:
                        continue  # fully covered by the k starter tiles
                    raw = singles.tile([128, half], dt.float32, name=f"{nm}raw{part}")
                    raws[(nm, part)] = raw
                    eng.dma_start(raw[:], src[:, csl])
            # bulk cast pieces, emitted lazily inside chunk 0's windows
            # (ordered by first use; k is fully needed by window 8, the
            # second q t-chunk only at window 16)
            cast_queue = []
            for nm, r_dst, lo, hi in (
                ("k", k_r, 2048, 3072),
                ("k", k_r, 3072, 4096),
                ("q", q_r, 512, 1536),
                ("q", q_r, 1536, 2048),
                ("q", q_r, 2048, 3072),
                ("q", q_r, 3072, 4096),
            ):
                for s0 in range(lo, hi, 1024):
                    s1 = min(s0 + 1024, hi)
                    part = s0 // half
                    cast_queue.append(
                        (r_dst, s0, s1, raws[(nm, part)], s0 - half * part)
                    )

            def emit_casts(n):
                for _ in range(min(n, len(cast_queue))):
                    r_dst, s0, s1, raw, r0 = cast_queue.pop(0)
                    nc.vector.tensor_copy(
                        r_dst[:, s0:s1], raw[:, r0 : r0 + (s1 - s0)]
                    )

            # W_h[:, j, :] = [V^T ; ones] s-tile j: [128 s, 65] fp16,
            # built lazily (2 js per head per window) via PE transpose.
            ws = []
            for h in range(HPC):
                w_h = singles.tile([128, NJ, 65], dt.float16, tag=f"W{h}")
                nc.vector.memset(w_h[:, :, 64:65], 1.0)
                ws.append(w_h)
            tcur = [0] * HPC

            def emit_transposes(n):
                for h in range(HPC):
                    for j in range(tcur[h], min(tcur[h] + n, NJ)):
                        pt = stp.tile([128, 64], dt.float32, tag="st", name="pt")
                        chain_pe(
                            nc.tensor.transpose(
                                pt[:],
                                v_f[64 * h : 64 * h + 64, 128 * j : 128 * (j + 1)],
                                ident[64 * h : 64 * h + 64, 64 * h : 64 * h + 64],
                            )
                        )
                        nc.vector.tensor_copy(ws[h][:, j, 0:64], pt[:])
                    tcur[h] = min(tcur[h] + n, NJ)

            # Normalization is deferred ~2 chunks: the per-chunk tail is
            # just two copies (PSUM -> persistent SBUF), and the
            # reciprocal/broadcast/multiply/DMA for chunk i are emitted
            # mid-stream during chunk i+2, when every cross-engine
            # dependency is ~80us stale. Emitting them near the chunk
            # boundary lets the Tile scheduler order the DVE multiply
            # (waiting on a Pool broadcast) against Pool combines
            # (waiting on DVE tensor_scalars) into a cross-queue
            # near-deadlock costing ~10us of PE stall per chunk.
            oraw = [
                singles.tile([64, L], dt.float16, tag=f"oraw{h}", name=f"oraw{h}")
                for h in range(HPC)
            ]
            draw = [
                singles.tile([1, L], dt.float32, tag=f"draw{h}", name=f"draw{h}")
                for h in range(HPC)
            ]
            norm_q = []

            def emit_tail_copy(i, h, pv_ps):
                tsl = slice(NT * i, NT * (i + 1))
                # two copies free the PSUM bank; the denominator row goes
                # to a partition-0 tile (custom-op ucode paths mishandle
                # partition offsets)
                nc.vector.tensor_copy(draw[h][:, tsl], pv_ps[64:65, :])
                nc.vector.tensor_copy(oraw[h][:, tsl], pv_ps[0:64, :])
                norm_q.append((i, h))

            def emit_norm():
                i, h = norm_q.pop(0)
                tsl = slice(NT * i, NT * (i + 1))
                r_sb = tails.tile([1, NT], dt.float32, tag="r")
                nc.vector.reciprocal_approx_fast(r_sb[:], draw[h][:, tsl])
                r_bc = tails.tile([64, NT], dt.float32, tag="rbc")
                nc.gpsimd.partition_broadcast(r_bc[:], r_sb[:])
                o_sb = outp.tile([64, NT], dt.float32, tag="o")
                nc.vector.tensor_mul(o_sb[:], oraw[h][0:64, tsl], r_bc[:])
                nc.sync.dma_start(o_d[64 * h : 64 * h + 64, tsl], o_sb[:])

            def emit_exp(g, st_ap, ew_ap):
                if _dve_win(g):
                    i1 = i16p.tile([128, WSZ * NT], dt.int16, tag="i1")
                    nc.vector.tensor_scalar(
                        i1[:],
                        st_ap,
                        A_TS,
                        B_TS,
                        mybir.AluOpType.mult,
                        mybir.AluOpType.add,
                    )
                    i2 = i16p.tile([128, WSZ * NT], dt.int16, tag="i2")
                    nc.vector.tensor_scalar(
                        i2[:], i1[:], K_SHIFT, None, mybir.AluOpType.add
                    )
                    nc.gpsimd.tensor_tensor(
                        ew_ap,
                        i1[:].bitcast(dt.float16),
                        i2[:].bitcast(dt.float16),
                        mybir.AluOpType.add,
                    )
                else:
                    nc.scalar.activation(
                        ew_ap,
                        st_ap,
                        mybir.ActivationFunctionType.Exp,
                        scale=SCALE,
                    )

            def flush(pend):
                i, j0, cnt, ews, pvs = pend
                for h in range(HPC):
                    _pv(nc, pvs[h], ws[h], (ews[h], j0, cnt), chain_pe)
                if j0 + cnt == NJ:
                    for h in range(HPC):
                        emit_tail_copy(i, h, pvs[h])
                elif j0 == 8 * WSZ and len(norm_q) >= HPC:
                    # mid-chunk: normalize the previous chunk
                    # (dependencies ~4 windows stale by now)
                    for _ in range(HPC):
                        emit_norm()

            # Global software pipeline over (t-chunk, window): QK(g) and
            # exp(g) are emitted PV_LAG windows ahead of PV(g-PV_LAG) so
            # the slower DVE+Pool exp chain never stalls the PE.
            pend_q = []
            gwin = [0]
            for _rep in range(repeat):
                for i in range(TCH):
                    tsl = slice(NT * i, NT * (i + 1))
                    pvs = [
                        pvp.tile([65, NT], dt.float32, tag="pv", name=f"pv{h}")
                        for h in range(HPC)
                    ]
                    for j0, cnt in windows:
                        if cast_queue and gwin[0] in (2, 4, 6, 8, 10, 12):
                            emit_casts(1)
                        # PV of the lagged window goes FIRST in the PE
                        # chain: QK(g) blocks on st-bank recycling (exp
                        # of g-1.5), and the in-order PE queue would
                        # block the ready PV work behind it.
                        if len(pend_q) > PV_LAG:
                            flush(pend_q.pop(0))
                        sts = [
                            stp.tile(
                                [128, WSZ * NT], dt.float32, tag="st", name=f"st{h}"
                            )
                            for h in range(HPC)
                        ]
                        # interleave heads so row-packed QK pairs overlap
                        for jj in range(cnt):
                            j = j0 + jj
                            for h in range(HPC):
                                hsl = slice(64 * h, 64 * h + 64)
                                chain_pe(
                                    nc.tensor.matmul(
                                        sts[h][:, NT * jj : NT * (jj + 1)],
                                        k_r[hsl, 128 * j : 128 * (j + 1)],
                                        q_r[hsl, tsl],
                                        start=True,
                                        stop=True,
                                        tile_position=(64 * h, 0),
                                    )
                                )
                        if tcur[0] < NJ:
                            emit_transposes(WSZ)
                        ews = []
                        for h in range(HPC):
                            ew = expp.tile([128, WSZ * NT], dt.float16, tag="ew")
                            emit_exp(
                                gwin[0] * HPC + h,
                                sts[h][:, 0 : cnt * NT],
                                ew[:, 0 : cnt * NT],
                            )
                            ews.append(ew)
                        gwin[0] += 1
                        pend_q.append((i, j0, cnt, ews, pvs))
            while pend_q:
                flush(pend_q.pop(0))
            while norm_q:
                emit_norm()

    nc.compile()
    return nc


def _pv(nc, pv_ps, w_h, pending, chain_pe):
    ew, j0, cnt = pending
    for jj in range(cnt):
        j = j0 + jj
        chain_pe(
            nc.tensor.matmul(
                pv_ps[:],
                w_h[:, j, :],
                ew[:, NT * jj : NT * (jj + 1)],
                start=(j == 0),
                stop=(j == NJ - 1),
            )
        )


def _get_nc(repeat: int = 1):
    if repeat not in _nc_cache:
        _nc_cache[repeat] = _build(repeat)
    return _nc_cache[repeat]


def kernel(qkv: np.ndarray, _repeat: int = 1) -> np.ndarray:
    qkv = np.asarray(qkv)
    bs, width, length = qkv.shape
    assert (bs, width, length) == (BS, 3 * NUM_HEADS * CH, L), qkv.shape
    hw = NUM_HEADS * CH

    nc = _get_nc(_repeat)
    in_maps = []
    for c in range(NCORES):
        qs, ks, vs = [], [], []
        for i in range(HPC):
            bh = c * HPC + i
            b, h = bh // NUM_HEADS, bh % NUM_HEADS
            qs.append(qkv[b, h * CH : (h + 1) * CH, :])
            ks.append(qkv[b, hw + h * CH : hw + (h + 1) * CH, :])
            vs.append(qkv[b, 2 * hw + h * CH : 2 * hw + (h + 1) * CH, :])
        in_maps.append(
            {
                "q": np.ascontiguousarray(np.concatenate(qs, axis=0)),
                "k": np.ascontiguousarray(np.concatenate(ks, axis=0)),
                "v": np.ascontiguousarray(np.concatenate(vs, axis=0)),
            }
        )

    res = run_bass_kernel_spmd(nc, in_maps, list(range(NCORES)))

    out = np.empty((BS, hw, L), np.float32)
    for c in range(NCORES):
        oc = res.results[c]["o"]
        for i in range(HPC):
            bh = c * HPC + i
            b, h = bh // NUM_HEADS, bh % NUM_HEADS
            out[b, h * CH : (h + 1) * CH, :] = oc[i * CH : (i + 1) * CH]
    return out


# revision 26
# speedup vs baseline: 1.3469x; 1.3469x over previous
"""Multi-head attention (bs=2, heads=8, ch=64, len=4096) on 8 Trainium2 cores.

Sharding: bs*heads = 16 head-problems, 2 per core (head/data parallel,
no cross-core communication).

Per-core algorithm (per head, seq len L=4096, ch=64):
  - S^T tiles: st[s,t] = sum_c K[c,s] Q[c,t] on PE in fp32r (no input
    casts; exact), heads packed in array row-halves via tile_position.
  - softmax exp: split across TWO engines to break the ScalarE-only
    roofline (33.5M exps/core = 218us at 1 elem/lane/cycle):
      * ~73% of windows: ScalarE ACT exp (exact, fp32->fp16)
      * ~27% of windows: DVE+Pool 2-phase Schraudolph bit-trick:
          i1 = rint(st*A + B) int16      (DVE tensor_scalar, PSUM read)
          i2 = i1 + K_SHIFT              (DVE tensor_scalar, int16)
          ew = bc16(i1) + bc16(i2)       (Pool tensor_tensor add, fp16)
        With K_SHIFT=-504, phase-2 lands half an octave off with weight
        2^-0.5, flattening the piecewise-linear 2^u error sawtooth:
        rel err std 0.54% (vs 1.77% single-phase), mean calibrated ~0
        via C_CAL so ACT-exact and DVE-approx windows agree in scale.
  - PV with folded denominator: lhsT = [V^T ; ones] fp16, accumulated
    over 32 s-tiles into PSUM [65, 512]; row 64 = softmax denominator.
  - normalize: broadcast denominator FIRST (Pool partition_broadcast),
    then wide [64,512] DVE reciprocal + Pool multiply (a [1,512]
    single-partition reciprocal costs 3.3us on DVE; wide costs 0.7us).

Engine budget per core (steady state): ACT ~185us, DVE ~185us
(exp share + tails), Pool ~160us (combines + tails), PE ~164us@2.4GHz.
"""

import sys

sys.path.insert(0, "/opt/trn_rl_repo")

import numpy as np
from concourse import mybir, tile, bacc
from concourse.bass_utils import run_bass_kernel_spmd
from concourse.masks import make_identity
from concourse.tile_rust import add_dep_helper

dt = mybir.dt

NUM_HEADS = 8
BS = 2
CH = 64
L = 4096
NCORES = 8
HPC = BS * NUM_HEADS // NCORES  # heads per core = 2
NT = 512  # t-chunk (matmul moving dim / PSUM bank)
TCH = L // NT  # 8 t-chunks
NJ = L // 128  # 32 s-tiles
WSZ = 2  # s-tiles per exp window (2 PSUM banks; stp bufs=3 -> 6 banks)
SCALE = float(CH) ** -0.5  # logit scale, folded into ACT / A_TS

# 2-phase Schraudolph constants (calibrated: rel std 0.54%, mean ~0)
LOG2E = 1.4426950408889634
A_TS = SCALE * LOG2E * 1024.0
C_CAL = 853.0
B_TS = 15360.0 - C_CAL
K_SHIFT = -504

# window assignment: DVE path gets F_NUM of every F_DEN windows
F_NUM, F_DEN = 6, 64
PV_LAG = 4  # windows of slack between exp emission and PV consumption

_nc_cache: dict = {}


def _dve_win(g: int) -> bool:
    return (g * F_NUM) % F_DEN < F_NUM


def _build(repeat: int = 1):
    nc = bacc.Bacc("TRN2", target_bir_lowering=False)
    q_d = nc.dram_tensor("q", [HPC * CH, L], dt.float32, kind="ExternalInput")
    k_d = nc.dram_tensor("k", [HPC * CH, L], dt.float32, kind="ExternalInput")
    v_d = nc.dram_tensor("v", [HPC * CH, L], dt.float32, kind="ExternalInput")
    o_d = nc.dram_tensor("o", [HPC * CH, L], dt.float32, kind="ExternalOutput")

    windows = []
    j = 0
    while j < NJ:
        windows.append((j, min(WSZ, NJ - j)))
        j += WSZ

    # Chain every TensorE instruction in emission order (sync=False =
    # order-only, no semaphore): the Tile scheduler otherwise groups
    # same-row-group K=64 matmuls, which serialize (LDWEIGHTS cannot be
    # pulled ahead); the emitted order keeps A/B pairs overlapping.
    prev_pe = [None]

    def chain_pe(bi):
        if prev_pe[0] is not None:
            add_dep_helper(bi.ins, prev_pe[0].ins, sync=False, reason="pe order")
        prev_pe[0] = bi
        return bi

    with tile.TileContext(nc) as tc:
        with (
            tc.tile_pool(name="singles", bufs=1) as singles,
            tc.tile_pool(name="expw", bufs=12) as expp,
            tc.tile_pool(name="i16w", bufs=8) as i16p,
            tc.tile_pool(name="outp", bufs=5) as outp,
            tc.tile_pool(name="tails", bufs=4) as tails,
            tc.tile_pool(name="stp", bufs=3, space="PSUM") as stp,
            tc.tile_pool(name="pvp", bufs=2, space="PSUM") as pvp,
        ):
            ident = singles.tile([128, 128], dt.float32)
            make_identity(nc, ident[:])

            # K and Q packed on the partition axis (head h occupies
            # partitions 64h..64h+63), cast to fp16 for the PE. DMA and
            # cast in column pieces so the first QK matmuls start before
            # the full tensors arrive. All input DMAs move [128, N]
            # tiles ([64, N] transfers get half the SBUF port BW).
            k_r = singles.tile([128, L], dt.float16)
            q_r = singles.tile([128, L], dt.float16)
            qs0 = singles.tile([128, 512], dt.float32)
            ks0 = singles.tile([128, 1024], dt.float32)
            ks1 = singles.tile([128, 1024], dt.float32)
            # ALL input DMAs go on ONE queue in priority order: the
            # first-QK starters get the full HBM pipe (parallel queues
            # share bandwidth and delay the pipeline start ~10us), then
            # V (transposes from ~w2), the k tail (needed w8), then q.
            nc.sync.dma_start(qs0[:], q_d[:, 0:512])
            nc.sync.dma_start(ks0[:], k_d[:, 0:1024])
            nc.sync.dma_start(ks1[:], k_d[:, 1024:2048])
            nc.vector.tensor_copy(q_r[:, 0:512], qs0[:])
            nc.vector.tensor_copy(k_r[:, 0:1024], ks0[:])
            nc.vector.tensor_copy(k_r[:, 1024:2048], ks1[:])

            v_f = singles.tile([128, L], dt.float32)
            half = L // 2
            raws = {}
            for nm in ("q", "k"):
                for part in range(2):
                    if nm == "k" and part == 0:
                        continue  # fully covered by the k starter tiles
                    raws[(nm, part)] = singles.tile(
                        [128, half], dt.float32, name=f"{nm}raw{part}"
                    )
            nc.sync.dma_start(v_f[:, 0:2048], v_d[:, 0:2048])
            nc.sync.dma_start(raws[("k", 1)][:], k_d[:, half:L])
            nc.sync.dma_start(v_f[:, 2048:L], v_d[:, 2048:L])
            nc.sync.dma_start(raws[("q", 0)][:], q_d[:, 0:half])
            nc.sync.dma_start(raws[("q", 1)][:], q_d[:, half:L])
            # bulk cast pieces, emitted lazily inside chunk 0's windows
            # (ordered by first use; k is fully needed by window 8, the
            # second q t-chunk only at window 16)
            cast_queue = []
            for nm, r_dst, lo, hi in (
                ("k", k_r, 2048, 3072),
                ("k", k_r, 3072, 4096),
                ("q", q_r, 512, 1536),
                ("q", q_r, 1536, 2048),
                ("q", q_r, 2048, 3072),
                ("q", q_r, 3072, 4096),
            ):
                for s0 in range(lo, hi, 1024):
                    s1 = min(s0 + 1024, hi)
                    part = s0 // half
                    cast_queue.append(
                        (r_dst, s0, s1, raws[(nm, part)], s0 - half * part)
                    )

            def emit_casts(n):
                for _ in range(min(n, len(cast_queue))):
                    r_dst, s0, s1, raw, r0 = cast_queue.pop(0)
                    nc.vector.tensor_copy(
                        r_dst[:, s0:s1], raw[:, r0 : r0 + (s1 - s0)]
                    )

            # W_h[:, j, :] = [V^T ; ones] s-tile j: [128 s, 65] fp16,
            # built lazily (2 js per head per window) via PE transpose.
            ws = []
            for h in range(HPC):
                w_h = singles.tile([128, NJ, 65], dt.float16, tag=f"W{h}")
                nc.vector.memset(w_h[:, :, 64:65], 1.0)
                ws.append(w_h)
            tcur = [0] * HPC

            def emit_transposes(n):
                for h in range(HPC):
                    for j in range(tcur[h], min(tcur[h] + n, NJ)):
                        pt = stp.tile([128, 64], dt.float32, tag="st", name="pt")
                        chain_pe(
                            nc.tensor.transpose(
                                pt[:],
                                v_f[64 * h : 64 * h + 64, 128 * j : 128 * (j + 1)],
                                ident[64 * h : 64 * h + 64, 64 * h : 64 * h + 64],
                            )
                        )
                        nc.vector.tensor_copy(ws[h][:, j, 0:64], pt[:])
                    tcur[h] = min(tcur[h] + n, NJ)

            # Normalization is deferred ~2 chunks: the per-chunk tail is
            # just two copies (PSUM -> persistent SBUF), and the
            # reciprocal/broadcast/multiply/DMA for chunk i are emitted
            # mid-stream during chunk i+2, when every cross-engine
            # dependency is ~80us stale. Emitting them near the chunk
            # boundary lets the Tile scheduler order the DVE multiply
            # (waiting on a Pool broadcast) against Pool combines
            # (waiting on DVE tensor_scalars) into a cross-queue
            # near-deadlock costing ~10us of PE stall per chunk.
            oraw = [
                singles.tile([64, L], dt.float16, tag=f"oraw{h}", name=f"oraw{h}")
                for h in range(HPC)
            ]
            draw = [
                singles.tile([1, L], dt.float32, tag=f"draw{h}", name=f"draw{h}")
                for h in range(HPC)
            ]
            norm_q = []

            def emit_tail_copy(i, h, pv_ps):
                tsl = slice(NT * i, NT * (i + 1))
                # two copies free the PSUM bank; the denominator row goes
                # to a partition-0 tile (custom-op ucode paths mishandle
                # partition offsets)
                nc.vector.tensor_copy(draw[h][:, tsl], pv_ps[64:65, :])
                nc.vector.tensor_copy(oraw[h][:, tsl], pv_ps[0:64, :])
                norm_q.append((i, h))

            def emit_norm():
                i, h = norm_q.pop(0)
                tsl = slice(NT * i, NT * (i + 1))
                r_sb = tails.tile([1, NT], dt.float32, tag="r")
                nc.vector.reciprocal_approx_fast(r_sb[:], draw[h][:, tsl])
                r_bc = tails.tile([64, NT], dt.float32, tag="rbc")
                nc.gpsimd.partition_broadcast(r_bc[:], r_sb[:])
                o_sb = outp.tile([64, NT], dt.float32, tag="o")
                nc.vector.tensor_mul(o_sb[:], oraw[h][0:64, tsl], r_bc[:])
                nc.sync.dma_start(o_d[64 * h : 64 * h + 64, tsl], o_sb[:])

            def emit_exp(g, st_ap, ew_ap):
                if _dve_win(g):
                    i1 = i16p.tile([128, WSZ * NT], dt.int16, tag="i1")
                    nc.vector.tensor_scalar(
                        i1[:],
                        st_ap,
                        A_TS,
                        B_TS,
                        mybir.AluOpType.mult,
                        mybir.AluOpType.add,
                    )
                    i2 = i16p.tile([128, WSZ * NT], dt.int16, tag="i2")
                    nc.vector.tensor_scalar(
                        i2[:], i1[:], K_SHIFT, None, mybir.AluOpType.add
                    )
                    nc.gpsimd.tensor_tensor(
                        ew_ap,
                        i1[:].bitcast(dt.float16),
                        i2[:].bitcast(dt.float16),
                        mybir.AluOpType.add,
                    )
                else:
                    nc.scalar.activation(
                        ew_ap,
                        st_ap,
                        mybir.ActivationFunctionType.Exp,
                        scale=SCALE,
                    )

            def flush(pend):
                i, j0, cnt, ews, pvs = pend
                for h in range(HPC):
                    _pv(nc, pvs[h], ws[h], (ews[h], j0, cnt), chain_pe)
                if j0 + cnt == NJ:
                    for h in range(HPC):
                        emit_tail_copy(i, h, pvs[h])
                elif j0 in (6 * WSZ, 12 * WSZ) and len(norm_q) >= HPC:
                    # mid-chunk: normalize the previous chunk
                    # (dependencies ~4 windows stale by now)
                    for _ in range(HPC):
                        emit_norm()

            # Global software pipeline over (t-chunk, window): QK(g) and
            # exp(g) are emitted PV_LAG windows ahead of PV(g-PV_LAG) so
            # the slower DVE+Pool exp chain never stalls the PE.
            pend_q = []
            gwin = [0]
            for _rep in range(repeat):
                for i in range(TCH):
                    tsl = slice(NT * i, NT * (i + 1))
                    pvs = [
                        pvp.tile([65, NT], dt.float32, tag="pv", name=f"pv{h}")
                        for h in range(HPC)
                    ]
                    for j0, cnt in windows:
                        if cast_queue and gwin[0] in (2, 4, 6, 8, 10, 12):
                            emit_casts(1)
                        # PV of the lagged window goes FIRST in the PE
                        # chain: QK(g) blocks on st-bank recycling (exp
                        # of g-1.5), and the in-order PE queue would
                        # block the ready PV work behind it.
                        if len(pend_q) > PV_LAG:
                            flush(pend_q.pop(0))
                        sts = [
                            stp.tile(
                                [128, WSZ * NT], dt.float32, tag="st", name=f"st{h}"
                            )
                            for h in range(HPC)
                        ]
                        # interleave heads so row-packed QK pairs overlap
                        for jj in range(cnt):
                            j = j0 + jj
                            for h in range(HPC):
                                hsl = slice(64 * h, 64 * h + 64)
                                chain_pe(
                                    nc.tensor.matmul(
                                        sts[h][:, NT * jj : NT * (jj + 1)],
                                        k_r[hsl, 128 * j : 128 * (j + 1)],
                                        q_r[hsl, tsl],
                                        start=True,
                                        stop=True,
                                        tile_position=(64 * h, 0),
                                    )
                                )
                        if tcur[0] < NJ:
                            emit_transposes(WSZ)
                        ews = [None] * HPC
                        horder = range(HPC) if gwin[0] % 2 == 0 else range(HPC - 1, -1, -1)
                        for h in horder:
                            ew = expp.tile([128, WSZ * NT], dt.float16, tag="ew")
                            emit_exp(
                                gwin[0] * HPC + h,
                                sts[h][:, 0 : cnt * NT],
                                ew[:, 0 : cnt * NT],
                            )
                            ews[h] = ew
                        gwin[0] += 1
                        pend_q.append((i, j0, cnt, ews, pvs))
            while pend_q:
                flush(pend_q.pop(0))
            while norm_q:
                emit_norm()

    nc.compile()
    return nc


def _pv(nc, pv_ps, w_h, pending, chain_pe):
    ew, j0, cnt = pending
    for jj in range(cnt):
        j = j0 + jj
        chain_pe(
            nc.tensor.matmul(
                pv_ps[:],
                w_h[:, j, :],
                ew[:, NT * jj : NT * (jj + 1)],
                start=(j == 0),
                stop=(j == NJ - 1),
            )
        )


def _get_nc(repeat: int = 1):
    if repeat not in _nc_cache:
        _nc_cache[repeat] = _build(repeat)
    return _nc_cache[repeat]


def kernel(qkv: np.ndarray, _repeat: int = 1) -> np.ndarray:
    qkv = np.asarray(qkv)
    bs, width, length = qkv.shape
    assert (bs, width, length) == (BS, 3 * NUM_HEADS * CH, L), qkv.shape
    hw = NUM_HEADS * CH

    nc = _get_nc(_repeat)
    in_maps = []
    for c in range(NCORES):
        qs, ks, vs = [], [], []
        for i in range(HPC):
            bh = c * HPC + i
            b, h = bh // NUM_HEADS, bh % NUM_HEADS
            qs.append(qkv[b, h * CH : (h + 1) * CH, :])
            ks.append(qkv[b, hw + h * CH : hw + (h + 1) * CH, :])
            vs.append(qkv[b, 2 * hw + h * CH : 2 * hw + (h + 1) * CH, :])
        in_maps.append(
            {
                "q": np.ascontiguousarray(np.concatenate(qs, axis=0)),
                "k": np.ascontiguousarray(np.concatenate(ks, axis=0)),
                "v": np.ascontiguousarray(np.concatenate(vs, axis=0)),
            }
        )

    res = run_bass_kernel_spmd(nc, in_maps, list(range(NCORES)))

    out = np.empty((BS, hw, L), np.float32)
    for c in range(NCORES):
        oc = res.results[c]["o"]
        for i in range(HPC):
            bh = c * HPC + i
            b, h = bh // NUM_HEADS, bh % NUM_HEADS
            out[b, h * CH : (h + 1) * CH, :] = oc[i * CH : (i + 1) * CH]
    return out
